# revision 9
# baseline (speedup 1.0000x reference)
"""MMLorentzKG scoring kernel for 8 Trainium2 NeuronCores.

Strategy (model/vocab parallel, per sharding hint):
  - Entity table (the matmul "vocab" side) is sharded over the entity axis:
    each of the 8 cores owns 10000 of the 80000 entities, pre-transposed and
    cast to bf16 on the host so the core's [B,640] @ [640,10000] score matmul
    streams K=640 in five full 128-row chunks.
  - Queries / relation params / gathered lhs rows are replicated to all cores.
  - Each core computes the full lhs-side Lorentz math (rotate/boost/attention,
    tiny: [2000,320]) in fp32, builds the combined query matrix
    vab = [lhs*aux0 - lhs_im*aux1 | lhs*aux1 + lhs_im*aux0] in bf16,
    PE-transposes it per 128-column chunk, and accumulates the score slab
    [2000,10000] in PSUM over the 5 K-chunks.
  - reg0..reg4 are computed identically on every core (negligible); host takes
    core 0's copy. Host concatenates the 8 score slabs along the entity axis.
"""

import warnings

warnings.filterwarnings("ignore")

import numpy as np
import ml_dtypes

import concourse.bass as bass
import concourse.mybir as mybir
import concourse.tile as tile
from concourse import bacc
from concourse.bass_utils import run_bass_kernel_spmd
from concourse.masks import make_identity

N_CORES = 8
B = 2000
RANK = 64
EC = 5           # entity components per rank slot (1 time + 4 space)
D = RANK * EC    # 320
N_ENT = 80000
NSH = N_ENT // N_CORES   # 10000 entities per core
P = 128
NB = (B + P - 1) // P    # 16 B-tiles (last one 80 rows)
NT = 500                 # entity columns per PSUM tile (<=512 fp32 bank)
NNT = NSH // NT          # 20
KC = 5                   # K chunks of 128 (K = 2*D = 640)
SCALE = 1.0 / np.sqrt(RANK)
EPS = 1e-9

F32 = mybir.dt.float32
BF16 = mybir.dt.bfloat16
OP = mybir.AluOpType
AX = mybir.AxisListType
AF = mybir.ActivationFunctionType

_CACHE = {}


def _make_nc(b=B, nsh=NSH, n_cores=N_CORES):
    """Trace the bass program for batch `b` and per-core entity shard `nsh`."""
    NB = (b + P - 1) // P
    NNT = nsh // NT

    nc = bacc.Bacc("TRN2", target_bir_lowering=False, debug=False,
                   enable_asserts=False, num_devices=n_cores)

    def din(name, shape, dtype=F32):
        return nc.dram_tensor(name, shape, dtype, kind="ExternalInput").ap()

    def dout(name, shape, dtype=F32):
        return nc.dram_tensor(name, shape, dtype, kind="ExternalOutput").ap()

    ent_d = din("ent", [KC, P, nsh], BF16)        # per-core shard (K-major)
    x_d = din("lhs_re", [b, RANK, EC])
    xi_d = din("lhs_im", [b, D])
    rr_d = din("rrot", [b, RANK, 4])
    rb_d = din("rboost", [b, RANK, 4])
    at_d = din("atten_s", [b, D])                 # pre-scaled by 1/sqrt(RANK)
    a0_d = din("aux0", [b, D])
    a1_d = din("aux1", [b, D])
    hr_d = din("rhs_re", [b, D])
    hi_d = din("rhs_im", [b, D])

    sc_d = dout("scores", [b, nsh])
    r0_d = dout("reg0", [b, D])
    r1_d = dout("reg1", [b, D])
    r2_d = dout("reg2", [b, D])
    r3_d = dout("reg3", [b, RANK])
    r4_d = dout("reg4", [b, RANK])

    tt = nc.vector.tensor_tensor
    red = nc.vector.tensor_reduce
    act = nc.scalar.activation

    with tile.TileContext(nc) as tc:
        with tc.tile_pool(name="const", bufs=1) as cp, \
             tc.tile_pool(name="entp", bufs=1) as ep, \
             tc.tile_pool(name="io", bufs=2) as io, \
             tc.tile_pool(name="wk", bufs=2) as wk, \
             tc.tile_pool(name="vabp", bufs=2) as vabp, \
             tc.tile_pool(name="ltp", bufs=2) as ltp, \
             tc.tile_pool(name="outp", bufs=4) as outp, \
             tc.tile_pool(name="regp", bufs=2) as regp, \
             tc.tile_pool(name="pscore", bufs=4, space="PSUM") as pscore, \
             tc.tile_pool(name="ptrans", bufs=3, space="PSUM") as ptrans:

            ident = cp.tile([P, P], BF16)
            make_identity(nc, ident)

            # Resident entity shard: 5 chunks of [128, 10000] bf16 (20KB/part each)
            ent_k = []
            for k in range(KC):
                ek = ep.tile([P, nsh], BF16, tag=f"ent{k}")
                nc.sync.dma_start(out=ek, in_=ent_d[k])
                ent_k.append(ek)

            for bt in range(NB):
                p = min(P, b - bt * P)
                row = slice(bt * P, bt * P + p)

                x = io.tile([P, RANK, EC], F32, tag="x")
                nc.sync.dma_start(out=x[:p], in_=x_d[row])
                xi = io.tile([P, D], F32, tag="xi")
                nc.sync.dma_start(out=xi[:p], in_=xi_d[row])
                rr = io.tile([P, RANK, 4], F32, tag="rr")
                nc.sync.dma_start(out=rr[:p], in_=rr_d[row])
                rb = io.tile([P, RANK, 4], F32, tag="rb")
                nc.sync.dma_start(out=rb[:p], in_=rb_d[row])
                at = io.tile([P, D], F32, tag="at")
                nc.sync.dma_start(out=at[:p], in_=at_d[row])
                a0 = io.tile([P, D], F32, tag="a0")
                nc.sync.dma_start(out=a0[:p], in_=a0_d[row])
                a1 = io.tile([P, D], F32, tag="a1")
                nc.sync.dma_start(out=a1[:p], in_=a1_d[row])
                hr = io.tile([P, D], F32, tag="hr")
                nc.sync.dma_start(out=hr[:p], in_=hr_d[row])
                hi = io.tile([P, D], F32, tag="hi")
                nc.sync.dma_start(out=hi[:p], in_=hi_d[row])

                tcomp = x[:p, :, 0]          # [p, 64] time component
                s = x[:p, :, 1:5]            # [p, 64, 4] spatial components

                # ---- Lorentz rotate: rot = (t, quat_mul(normalize(rr), s)) ----
                qq = wk.tile([P, RANK, 4], F32, tag="qq")
                tt(qq[:p], rr[:p], rr[:p], op=OP.mult)
                r2n = wk.tile([P, RANK], F32, tag="r2n")
                red(r2n[:p], qq[:p], axis=AX.X, op=OP.add)      # sum rr^2 (reg3)
                rn = wk.tile([P, RANK], F32, tag="rn")
                nc.vector.tensor_scalar_add(rn[:p], r2n[:p], EPS)
                act(rn[:p], rn[:p], AF.Sqrt)                    # sqrt(.+eps)
                nc.vector.reciprocal(rn[:p], rn[:p])            # rsqrt
                q4 = wk.tile([P, RANK, 4], F32, tag="q4")
                tt(q4[:p], rr[:p], rn[:p].to_broadcast([p, RANK, 4]), op=OP.mult)

                rot = wk.tile([P, RANK, EC], F32, tag="rot")
                nc.vector.tensor_copy(rot[:p, :, 0], tcomp)
                qw, qx, qy, qz = (q4[:p, :, c] for c in range(4))
                pw, px, py, pz = (x[:p, :, c + 1] for c in range(4))
                tm = wk.tile([P, RANK], F32, tag="tm")
                # out_w = qw pw - qx px - qy py - qz pz
                ham = [
                    (1, [(qw, pw, OP.mult), (qx, px, OP.subtract),
                         (qy, py, OP.subtract), (qz, pz, OP.subtract)]),
                    # out_x = qw px + qx pw + qy pz - qz py
                    (2, [(qw, px, OP.mult), (qx, pw, OP.add),
                         (qy, pz, OP.add), (qz, py, OP.subtract)]),
                    # out_y = qw py - qx pz + qy pw + qz px
                    (3, [(qw, py, OP.mult), (qx, pz, OP.subtract),
                         (qy, pw, OP.add), (qz, px, OP.add)]),
                    # out_z = qw pz + qx py - qy px + qz pw
                    (4, [(qw, pz, OP.mult), (qx, py, OP.add),
                         (qy, px, OP.subtract), (qz, pw, OP.add)]),
                ]
                for comp, terms in ham:
                    dst = rot[:p, :, comp]
                    qf, pf, _ = terms[0]
                    tt(dst, qf, pf, op=OP.mult)
                    for qf, pf, accop in terms[1:]:
                        tt(tm[:p], qf, pf, op=OP.mult)
                        tt(dst, dst, tm[:p], op=accop)

                # ---- Lorentz boost ----
                vv = wk.tile([P, RANK, 4], F32, tag="vv")
                tt(vv[:p], rb[:p], rb[:p], op=OP.mult)
                v2 = wk.tile([P, RANK], F32, tag="v2")
                red(v2[:p], vv[:p], axis=AX.X, op=OP.add)       # sum rb^2 (reg4)
                vc = wk.tile([P, RANK], F32, tag="vc")
                nc.vector.tensor_scalar_min(vc[:p], v2[:p], 0.999)
                om = wk.tile([P, RANK], F32, tag="om")
                nc.vector.tensor_scalar(om[:p], vc[:p], -1.0, 1.0,
                                        op0=OP.mult, op1=OP.add)  # 1 - vn2
                gam = wk.tile([P, RANK], F32, tag="gam")
                act(gam[:p], om[:p], AF.Sqrt)
                nc.vector.reciprocal(gam[:p], gam[:p])          # gamma
                sv = wk.tile([P, RANK, 4], F32, tag="sv")
                tt(sv[:p], rb[:p], s, op=OP.mult)
                vs = wk.tile([P, RANK], F32, tag="vs")
                red(vs[:p], sv[:p], axis=AX.X, op=OP.add)       # v.s
                boo = wk.tile([P, RANK, EC], F32, tag="boo")
                tpv = wk.tile([P, RANK], F32, tag="tpv")
                tt(tpv[:p], tcomp, vs[:p], op=OP.add)
                tt(boo[:p, :, 0], gam[:p], tpv[:p], op=OP.mult)  # t2
                den = wk.tile([P, RANK], F32, tag="den")
                nc.vector.tensor_scalar_add(den[:p], vc[:p], EPS)
                nc.vector.reciprocal(den[:p], den[:p])
                gm1 = wk.tile([P, RANK], F32, tag="gm1")
                nc.vector.tensor_scalar_add(gm1[:p], gam[:p], -1.0)
                c2t = wk.tile([P, RANK], F32, tag="c2t")
                tt(c2t[:p], gm1[:p], vs[:p], op=OP.mult)
                tt(c2t[:p], c2t[:p], den[:p], op=OP.mult)
                coef = wk.tile([P, RANK], F32, tag="coef")
                tt(coef[:p], gam[:p], tcomp, op=OP.mult)
                tt(coef[:p], coef[:p], c2t[:p], op=OP.add)
                tmp4 = wk.tile([P, RANK, 4], F32, tag="tmp4")
                tt(tmp4[:p], rb[:p], coef[:p].to_broadcast([p, RANK, 4]), op=OP.mult)
                tt(boo[:p, :, 1:5], s, tmp4[:p], op=OP.add)

                rotf = rot[:p].rearrange("a b c -> a (b c)")     # [p, 320]
                boof = boo[:p].rearrange("a b c -> a (b c)")

                # ---- attention over {rot, boo} (2-way softmax = sigmoid) ----
                prod = wk.tile([P, D], F32, tag="prod")
                lr = wk.tile([P, 1], F32, tag="lr")
                lb = wk.tile([P, 1], F32, tag="lb")
                tt(prod[:p], rotf, at[:p], op=OP.mult)
                red(lr[:p], prod[:p], axis=AX.X, op=OP.add)
                tt(prod[:p], boof, at[:p], op=OP.mult)
                red(lb[:p], prod[:p], axis=AX.X, op=OP.add)
                dlt = wk.tile([P, 1], F32, tag="dlt")
                tt(dlt[:p], lr[:p], lb[:p], op=OP.subtract)
                wr = wk.tile([P, 1], F32, tag="wr")
                act(wr[:p], dlt[:p], AF.Sigmoid)
                wb = wk.tile([P, 1], F32, tag="wb")
                nc.vector.tensor_scalar(wb[:p], wr[:p], -1.0, 1.0,
                                        op0=OP.mult, op1=OP.add)
                lhs = wk.tile([P, D], F32, tag="lhs")
                tmpD = wk.tile([P, D], F32, tag="tmpD")
                nc.vector.tensor_scalar_mul(tmpD[:p], boof, wb[:p])
                nc.vector.scalar_tensor_tensor(lhs[:p], rotf, wr[:p], tmpD[:p],
                                               op0=OP.mult, op1=OP.add)

                # ---- vab = [lhs*aux0 - lhs_im*aux1 | lhs*aux1 + lhs_im*aux0] ----
                vab = vabp.tile([P, 2 * D], BF16, tag="vab")
                t1 = wk.tile([P, D], F32, tag="t1")
                tt(t1[:p], lhs[:p], a0[:p], op=OP.mult)
                tt(tmpD[:p], xi[:p], a1[:p], op=OP.mult)
                tt(vab[:p, 0:D], t1[:p], tmpD[:p], op=OP.subtract)
                tt(t1[:p], lhs[:p], a1[:p], op=OP.mult)
                tt(tmpD[:p], xi[:p], a0[:p], op=OP.mult)
                tt(vab[:p, D:2 * D], t1[:p], tmpD[:p], op=OP.add)

                # ---- transpose vab into K-chunk layout [128, 5, p] ----
                lhsT = ltp.tile([P, KC, P], BF16, tag="lhsT")
                for j in range(KC):
                    ptr = ptrans.tile([P, P], BF16, tag="ptr")
                    nc.tensor.transpose(ptr[:, :p],
                                        vab[:p, j * P:(j + 1) * P],
                                        ident[:p, :p])
                    nc.vector.tensor_copy(lhsT[:, j, :p], ptr[:, :p])

                # ---- score matmuls: [p,640] @ [640, 10000] in 500-col tiles ----
                # Two PSUM tiles share one [128,1000] out tile (4KB DMA rows);
                # PSUM->SBUF copies alternate DVE/ACT to keep DVE off the
                # critical path.
                for nt2 in range(NNT // 2):
                    ot = outp.tile([P, 2 * NT], F32, tag="ot")
                    for half in range(2):
                        nt = nt2 * 2 + half
                        col = slice(nt * NT, (nt + 1) * NT)
                        ps = pscore.tile([P, NT], F32, tag="ps")
                        for k in range(KC):
                            nc.tensor.matmul(ps[:p], lhsT[:, k, :p],
                                             ent_k[k][:, col],
                                             start=(k == 0), stop=(k == KC - 1))
                        dst = ot[:p, half * NT:(half + 1) * NT]
                        if half == 0:
                            nc.vector.tensor_copy(dst, ps[:p])
                        else:
                            nc.scalar.activation(dst, ps[:p], AF.Copy)
                    nc.sync.dma_start(
                        out=sc_d[row, nt2 * 2 * NT:(nt2 + 1) * 2 * NT],
                        in_=ot[:p])

                # ---- regularizers (elementwise on the otherwise-idle GpSimd) ----
                xf = x[:p].rearrange("a b c -> a (b c)")        # [p, 320]
                gtt = nc.gpsimd.tensor_tensor
                sq = regp.tile([P, D], F32, tag="sq")
                gsc = regp.tile([P, D], F32, tag="gsc")
                r0t = regp.tile([P, D], F32, tag="r0t")
                gtt(sq[:p], xf, xf, op=OP.mult)
                gtt(gsc[:p], xi[:p], xi[:p], op=OP.mult)
                gtt(sq[:p], sq[:p], gsc[:p], op=OP.add)
                act(r0t[:p], sq[:p], AF.Sqrt)
                nc.sync.dma_start(out=r0_d[row], in_=r0t[:p])

                r1t = regp.tile([P, D], F32, tag="r1t")
                gtt(sq[:p], a0[:p], a0[:p], op=OP.mult)
                gtt(gsc[:p], a1[:p], a1[:p], op=OP.mult)
                gtt(sq[:p], sq[:p], gsc[:p], op=OP.add)
                act(r1t[:p], sq[:p], AF.Sqrt)
                nc.sync.dma_start(out=r1_d[row], in_=r1t[:p])

                r2t = regp.tile([P, D], F32, tag="r2t")
                gtt(sq[:p], hr[:p], hr[:p], op=OP.mult)
                gtt(gsc[:p], hi[:p], hi[:p], op=OP.mult)
                gtt(sq[:p], sq[:p], gsc[:p], op=OP.add)
                act(r2t[:p], sq[:p], AF.Sqrt, scale=1.0 / 9.0)  # sqrt(x/9)=sqrt(x)/3
                nc.sync.dma_start(out=r2_d[row], in_=r2t[:p])

                r3t = regp.tile([P, RANK], F32, tag="r3t")
                act(r3t[:p], r2n[:p], AF.Sqrt)
                nc.sync.dma_start(out=r3_d[row], in_=r3t[:p])

                r4t = regp.tile([P, RANK], F32, tag="r4t")
                act(r4t[:p], v2[:p], AF.Sqrt)
                nc.sync.dma_start(out=r4_d[row], in_=r4t[:p])

    return nc


def _build_program():
    if "nc" in _CACHE:
        return _CACHE["nc"]
    nc = _make_nc()
    nc.compile()
    _CACHE["nc"] = nc
    return nc


def kernel(queries, stru_entities, stru_rel_rotate, stru_rel_boosts,
           stru_atten, stru_rel_aux):
    q = np.asarray(queries)
    E = np.asarray(stru_entities, dtype=np.float32)
    Rr = np.asarray(stru_rel_rotate, dtype=np.float32)
    Rb = np.asarray(stru_rel_boosts, dtype=np.float32)
    At = np.asarray(stru_atten, dtype=np.float32)
    Ax = np.asarray(stru_rel_aux, dtype=np.float32)

    h, r, t = q[:, 0], q[:, 1], q[:, 2]
    slhs = E[h]                                   # [B, 128, 5]
    srhs = E[t]
    common = {
        "lhs_re": np.ascontiguousarray(slhs[:, :RANK, :]),
        "lhs_im": np.ascontiguousarray(slhs[:, RANK:, :]).reshape(B, D),
        "rhs_re": np.ascontiguousarray(srhs[:, :RANK, :]).reshape(B, D),
        "rhs_im": np.ascontiguousarray(srhs[:, RANK:, :]).reshape(B, D),
        "rrot": np.ascontiguousarray(Rr[r]),
        "rboost": np.ascontiguousarray(Rb[r]),
        "atten_s": np.ascontiguousarray(At[r][:, 0, :] * np.float32(SCALE)),
        "aux0": np.ascontiguousarray(Ax[r][:, 0, :]),
        "aux1": np.ascontiguousarray(Ax[r][:, 1, :]),
    }

    to_re = E[:, :RANK, :].reshape(N_ENT, D)
    to_im = E[:, RANK:, :].reshape(N_ENT, D)
    comb = np.concatenate([to_re, to_im], axis=1)          # [N_ENT, 640]
    combT = np.ascontiguousarray(comb.T).astype(ml_dtypes.bfloat16)  # [640, N_ENT]

    in_maps = []
    for c in range(N_CORES):
        ent = np.ascontiguousarray(combT[:, c * NSH:(c + 1) * NSH])
        in_maps.append({**common, "ent": ent.reshape(KC, P, NSH)})

    nc = _build_program()
    res = run_bass_kernel_spmd(nc, in_maps, core_ids=list(range(N_CORES)))
    scores = np.concatenate(
        [res.results[c]["scores"] for c in range(N_CORES)], axis=1)
    r0 = res.results[0]
    return (scores, r0["reg0"], r0["reg1"], r0["reg2"], r0["reg3"], r0["reg4"])


if __name__ == "__main__":
    rng = np.random.default_rng(0)
    inputs = {
        "queries": rng.integers(0, 500, size=(B, 3)).astype(np.int32),
        "stru_entities": (rng.standard_normal((N_ENT, 2 * RANK, EC)) * 1e-3).astype(np.float32),
        "stru_rel_rotate": (rng.standard_normal((500, RANK, 4)) * 1e-3).astype(np.float32),
        "stru_rel_boosts": (rng.standard_normal((500, RANK, 4)) * 1e-3).astype(np.float32),
        "stru_atten": (rng.standard_normal((500, 1, D)) * 1e-3).astype(np.float32),
        "stru_rel_aux": (rng.standard_normal((500, 2, D)) * 1e-3).astype(np.float32),
    }
    outs = kernel(**inputs)
    for o in outs:
        print(o.shape, o.dtype, float(np.abs(o).max()))
